# revision 32
# baseline (speedup 1.0000x reference)
"""Chamfer loss (with color) Trainium2 Bass kernel.

Strategy (8 NeuronCores, SPMD, no collectives):
  - core c handles batch b=c//4, row-shard s=c%4 (2048 rows of each direction).
  - direction 0: x-shard rows vs all y (x->y mins); direction 1: y-shard rows
    vs all x (y->x mins).  Each core's row results are complete, so the only
    cross-core step is summing 8 small partial vectors on the host.
  - pairwise squared distances via PE matmul in "3-way bf16 split" form:
    each fp32 operand f is split exactly into h+m+l (three bf16 pieces of the
    24-bit mantissa).  Keeping product terms down to ~2^-24 gives fp32-grade
    d2 with bf16 matmul throughput (1 cyc/row vs 4 for fp32).  K = 24 rows:
      x-side: [h h h, h h h, m m m, m m m, h h h, l l l, nh nm nl, 1 1 1]
      y-side: [H H H, M M M, H H H, M M M, L L L, H H H, 1  1  1, NH NM NL]
    where y-side coords carry the -2 factor and n*/N* are splits of the
    squared norms.
  - per 128-row block: PE writes d2 into PSUM [128,2048] chunks; DVE reduces
    per-64-col subtile mins; the global row min picks a subtile, whose 64
    y-points are fetched by dma_gather and re-scored exactly on-chip
    ((y-r)^2 difference form, fp32).  That yields the exact min value and the
    nearest neighbor's color without ever extracting a global argmin index.
"""

import sys

if "/opt/trn_rl_repo" not in sys.path:
    sys.path.insert(0, "/opt/trn_rl_repo")

import numpy as np

ALPHA = 0.5
B, N, M, D = 2, 8192, 8192, 6
N_CORES = 8
SHARDS_PER_BATCH = 4


# ---------------------------------------------------------------- host-side
def _split3(a):
    """Exact 3-way bf16 split of fp32 array: a ~= h + m + l (fp32 views)."""
    import ml_dtypes

    bf = ml_dtypes.bfloat16
    h = a.astype(bf).astype(np.float32)
    r1 = (a - h).astype(np.float32)
    m_ = r1.astype(bf).astype(np.float32)
    r2 = (r1 - m_).astype(np.float32)
    l_ = r2.astype(bf).astype(np.float32)
    return h, m_, l_


def _stat_feats(pts):
    """x-side (stationary) features [24, n] bf16 from raw points [n, 6]."""
    import ml_dtypes

    c = pts[:, :3].astype(np.float32)
    n2 = (c.astype(np.float32) ** 2).sum(1, dtype=np.float32).astype(np.float32)
    xh, xm, xl = _split3(c)  # [n,3]
    nh, nm, nl = _split3(n2)  # [n]
    ones = np.ones_like(n2)
    rows = [
        xh[:, 0], xh[:, 1], xh[:, 2],
        xh[:, 0], xh[:, 1], xh[:, 2],
        xm[:, 0], xm[:, 1], xm[:, 2],
        xm[:, 0], xm[:, 1], xm[:, 2],
        xh[:, 0], xh[:, 1], xh[:, 2],
        xl[:, 0], xl[:, 1], xl[:, 2],
        nh, nm, nl, ones, ones, ones,
    ]
    return np.ascontiguousarray(np.stack(rows, 0)).astype(ml_dtypes.bfloat16)


def _mov_feats(pts):
    """y-side (moving) features [24, m] bf16: coords carry the -2 factor."""
    import ml_dtypes

    c = pts[:, :3].astype(np.float32)
    n2 = (c.astype(np.float32) ** 2).sum(1, dtype=np.float32).astype(np.float32)
    yh, ym, yl = _split3((-2.0 * c).astype(np.float32))
    NH, NM, NL = _split3(n2)
    ones = np.ones_like(n2)
    rows = [
        yh[:, 0], yh[:, 1], yh[:, 2],
        ym[:, 0], ym[:, 1], ym[:, 2],
        yh[:, 0], yh[:, 1], yh[:, 2],
        ym[:, 0], ym[:, 1], ym[:, 2],
        yl[:, 0], yl[:, 1], yl[:, 2],
        yh[:, 0], yh[:, 1], yh[:, 2],
        ones, ones, ones, NH, NM, NL,
    ]
    return np.ascontiguousarray(np.stack(rows, 0)).astype(ml_dtypes.bfloat16)


# dma_gather unwraps indices as idx[j] = A[8*(j%16) + j//16]; feeding the
# per-block index vector A in natural order therefore lands row r(j)'s
# segment on partition j, with r(j) = 8*(j%16) + j//16 (a fixed 16x8
# transpose permutation).  Per-row side data must use the same permutation.
_GATHER_PERM = np.array([8 * (j % 16) + j // 16 for j in range(128)])


def _rows_t(pts):
    """[R, 6] -> [128, (R//128)*6] fp32: partition j holds rows perm[j],
    128+perm[j], ... matching dma_gather's output placement."""
    R = pts.shape[0]
    nb = R // 128
    return np.ascontiguousarray(
        pts.reshape(nb, 128, 6)[:, _GATHER_PERM].transpose(1, 0, 2).reshape(128, nb * 6)
    ).astype(np.float32)


# ------------------------------------------------------------- bass program
def build_program(R=2048, C=8192, W=64, CH=2048):
    """One SPMD program: R shard rows per direction, C columns, subtile W."""
    import concourse.mybir as mybir
    from concourse import bacc
    from concourse.tile import TileContext

    fp32 = mybir.dt.float32
    bf16 = mybir.dt.bfloat16
    i16 = mybir.dt.int16
    i32 = mybir.dt.int32
    X = mybir.AxisListType.X
    OP = mybir.AluOpType
    AF = mybir.ActivationFunctionType

    NB = R // 128            # row blocks per direction
    S = C // W               # subtiles per row
    CH = min(CH, C)          # psum chunk width
    NCHUNK = C // CH
    TPC = CH // 512          # matmuls per chunk
    SPC = CH // W            # subtile mins per chunk

    nc = bacc.Bacc("TRN2")

    ins = {}
    for d in range(2):
        ins[f"stat{d}"] = nc.dram_tensor(f"stat{d}", [24, R], bf16, kind="ExternalInput")
        ins[f"mov{d}"] = nc.dram_tensor(f"mov{d}", [24, C], bf16, kind="ExternalInput")
        ins[f"rows{d}"] = nc.dram_tensor(f"rows{d}", [128, NB * 6], fp32, kind="ExternalInput")
        ins[f"gsrc{d}"] = nc.dram_tensor(f"gsrc{d}", [C, 6], fp32, kind="ExternalInput")
    ins["xnormsq"] = nc.dram_tensor("xnormsq", [128, C // 128], fp32, kind="ExternalInput")
    ins["iotaf"] = nc.dram_tensor("iotaf", [128, S], fp32, kind="ExternalInput")
    partials = nc.dram_tensor("partials", [1, 8], fp32, kind="ExternalOutput")

    with TileContext(nc) as tc:
        with (
            tc.tile_pool(name="const", bufs=1) as constp,
            tc.tile_pool(name="feats", bufs=1) as featp,
            tc.tile_pool(name="psum", bufs=2, space="PSUM") as psump,
            tc.tile_pool(name="mins", bufs=4) as minp,
            tc.tile_pool(name="seg", bufs=3) as segp,
            tc.tile_pool(name="acc", bufs=2) as accp,
            tc.tile_pool(name="idxdram", bufs=8, space="DRAM") as idxdp,
            tc.tile_pool(name="small", bufs=8) as smallp,
        ):
            # constants
            iota_f = constp.tile([128, S], fp32)
            nc.sync.dma_start(iota_f[:], ins["iotaf"][:])

            # final per-direction scalars end up here, then one DMA out
            staging = constp.tile([1, 8], fp32)
            nc.vector.memset(staging[:], 0.0)

            st = {}
            for d in range(2):
                st[d] = {}
                stat_sb = featp.tile([24, R], bf16, tag=f"stat{d}")
                nc.sync.dma_start(stat_sb[:], ins[f"stat{d}"][:])
                mov_sb = featp.tile([24, C], bf16, tag=f"mov{d}")
                nc.sync.dma_start(mov_sb[:], ins[f"mov{d}"][:])
                rows_sb = featp.tile([128, NB, 6], fp32, tag=f"rows{d}")
                nc.sync.dma_start(rows_sb[:], ins[f"rows{d}"][:].rearrange("p (b k) -> p b k", k=6))
                negrows = featp.tile([128, NB, 6], fp32, tag=f"negrows{d}")
                nc.scalar.mul(negrows[:], rows_sb[:], -1.0)
                st[d]["stat_sb"], st[d]["mov_sb"] = stat_sb, mov_sb
                st[d]["rows_sb"], st[d]["negrows"] = rows_sb, negrows
                st[d]["smin_acc"] = accp.tile([128, NB], fp32, tag=f"smin{d}", name=f"smin{d}")
                st[d]["cnt_acc"] = accp.tile([128, NB], fp32, tag=f"cnt{d}", name=f"cnt{d}")
                st[d]["csel_acc"] = accp.tile([128, 3, NB], fp32, tag=f"csel{d}", name=f"csel{d}")
                st[d]["yseg_all"] = featp.tile([128, NB, W * 6], fp32, tag=f"yseg{d}", name=f"yseg{d}")

            # ---- phase 1 per dir: matmul -> subtile-min pipeline, then
            # batched subtile pick + gathers (overlaps next dir's matmuls) ----
            for d in range(2):
                stat_sb, mov_sb = st[d]["stat_sb"], st[d]["mov_sb"]
                submins_all = featp.tile(
                    [128, NB, S], fp32, tag=f"submins{d}", name=f"submins{d}"
                )
                st[d]["submins_all"] = submins_all
                for blk in range(NB):
                    for ch in range(NCHUNK):
                        ps = psump.tile([128, CH], fp32, tag="d2")
                        for t in range(TPC):
                            nc.tensor.matmul(
                                ps[:, t * 512:(t + 1) * 512],
                                stat_sb[:, blk * 128:(blk + 1) * 128],
                                mov_sb[:, ch * CH + t * 512: ch * CH + (t + 1) * 512],
                                start=True, stop=True,
                            )
                        nc.vector.tensor_reduce(
                            submins_all[:, blk, ch * SPC:(ch + 1) * SPC],
                            ps[:].rearrange("p (s w) -> p s w", w=W),
                            axis=X, op=OP.min,
                        )
                yseg_all = st[d]["yseg_all"]
                gsrc_seg = ins[f"gsrc{d}"][:].rearrange("(s w) k -> s (w k)", w=W)
                gmin_all = accp.tile([128, NB], fp32, tag=f"gmin{d}", name=f"gmin{d}")
                nc.vector.tensor_reduce(gmin_all[:], submins_all[:], axis=X, op=OP.min)
                # first matching subtile: non-matches get +1e9, then
                # min-reduce over (penalty + index).  A plain sum of matching
                # indices corrupts rows whose quantized minima tie across
                # subtiles (~0.4% of rows on this data).
                pen = featp.tile([128, NB, S], fp32, tag=f"pen{d}", name=f"pen{d}")
                nc.vector.tensor_tensor(
                    out=pen[:], in0=submins_all[:],
                    in1=gmin_all[:].unsqueeze(2).to_broadcast([128, NB, S]),
                    op=OP.not_equal,
                )
                nc.vector.tensor_scalar_mul(pen[:], pen[:], 1e9)
                nc.vector.tensor_tensor(
                    out=pen[:], in0=pen[:],
                    in1=iota_f[:].unsqueeze(1).to_broadcast([128, NB, S]),
                    op=OP.add,
                )
                sstar_all = accp.tile([128, NB], fp32, tag=f"sstar{d}", name=f"sstar{d}")
                nc.vector.tensor_reduce(sstar_all[:], pen[:], axis=X, op=OP.min)
                nc.vector.tensor_scalar_min(sstar_all[:], sstar_all[:], float(S - 1))
                sstar_i = accp.tile([128, NB], i16, tag=f"sstar_i{d}", name=f"sstar_i{d}")
                nc.vector.tensor_copy(sstar_i[:], sstar_all[:])
                # 8 DRAM replicas of ALL blocks' index vectors, then one
                # strided read per block rebuilds the 16-wrapped layout
                # dma_gather expects (see _GATHER_PERM).
                idxd = idxdp.tile([8, 128, NB], i16, tag=f"idxd{d}", name=f"idxd{d}")
                for r in range(8):
                    nc.sync.dma_start(idxd[r], sstar_i[:])
                idxv = idxd[:].rearrange("a (bp s) nb -> a bp s nb", s=8)
                for blk in range(NB):
                    idx_sb = smallp.tile([128, 8], i16, tag="idx_sb")
                    nc.sync.dma_start(idx_sb[:], idxv[:, :, :, blk])
                    nc.gpsimd.dma_gather(
                        out_ap=yseg_all[:, blk:blk + 1, :], in_ap=gsrc_seg,
                        idxs_ap=idx_sb[:],
                        num_idxs=128, num_idxs_reg=128, elem_size=W * 6,
                    )

            # ---- phase 2 (both dirs): rescore all gathered segments ----
            for d in range(2):
                yseg_all, negrows = st[d]["yseg_all"], st[d]["negrows"]
                rows_sb = st[d]["rows_sb"]
                smin_acc, cnt_acc = st[d]["smin_acc"], st[d]["cnt_acc"]
                csel_acc = st[d]["csel_acc"]
                ysegv = yseg_all[:].rearrange("p b (w k) -> p b w k", k=6)
                sq_all = featp.tile([128, NB, W, 3], fp32, tag=f"sq{d}")
                for blk in range(NB):
                    for dd in range(3):
                        nc.scalar.activation(
                            sq_all[:, blk, :, dd], ysegv[:, blk, :, dd], AF.Square,
                            bias=negrows[:, blk, dd:dd + 1], scale=1.0,
                        )
                d2seg = featp.tile([128, NB, W], fp32, tag=f"d2seg{d}")
                nc.vector.tensor_reduce(d2seg[:], sq_all[:], axis=X, op=OP.add)
                nc.vector.tensor_reduce(smin_acc[:], d2seg[:], axis=X, op=OP.min)
                mask = featp.tile([128, NB, W], fp32, tag=f"mask{d}")
                nc.vector.tensor_tensor(
                    out=mask[:], in0=d2seg[:],
                    in1=smin_acc[:].unsqueeze(2).to_broadcast([128, NB, W]),
                    op=OP.is_equal,
                )
                nc.vector.tensor_reduce(cnt_acc[:], mask[:], axis=X, op=OP.add)
                tmp3 = featp.tile([128, NB, W], fp32, tag=f"tmp3{d}")
                for dd in range(3):
                    nc.vector.tensor_tensor(
                        out=tmp3[:], in0=mask[:], in1=ysegv[:, :, :, 3 + dd], op=OP.mult
                    )
                    nc.vector.tensor_reduce(csel_acc[:, dd, :], tmp3[:], axis=X, op=OP.add)

                # ---- per-direction finishers ----
                # geo: sum over rows of sqrt(max(min_d2, 0))
                nc.vector.tensor_scalar_max(smin_acc[:], smin_acc[:], 0.0)
                sqg = accp.tile([128, NB], fp32, tag=f"sqg{d}")
                nc.scalar.sqrt(sqg[:], smin_acc[:])
                geo_vec = accp.tile([128, 1], fp32, tag=f"geov{d}")
                nc.vector.tensor_reduce(geo_vec[:], sqg[:], axis=X, op=OP.add)
                # color: |color_x - sel_color/cnt| per row, then sum
                rec = accp.tile([128, NB], fp32, tag=f"rec{d}")
                nc.vector.reciprocal(rec[:], cnt_acc[:])
                csq = accp.tile([128, NB], fp32, tag=f"csq{d}")
                tmp = accp.tile([128, NB], fp32, tag=f"tmp{d}")
                for dd in range(3):
                    nc.vector.tensor_tensor(
                        out=tmp[:], in0=csel_acc[:, dd, :], in1=rec[:], op=OP.mult
                    )
                    nc.vector.tensor_tensor(
                        out=tmp[:], in0=tmp[:], in1=rows_sb[:, :, 3 + dd], op=OP.subtract
                    )
                    if dd == 0:
                        nc.vector.tensor_tensor(out=csq[:], in0=tmp[:], in1=tmp[:], op=OP.mult)
                    else:
                        nc.vector.tensor_tensor(out=tmp[:], in0=tmp[:], in1=tmp[:], op=OP.mult)
                        nc.vector.tensor_tensor(out=csq[:], in0=csq[:], in1=tmp[:], op=OP.add)
                cdist = accp.tile([128, NB], fp32, tag=f"cdist{d}")
                nc.scalar.sqrt(cdist[:], csq[:])
                col_vec = accp.tile([128, 1], fp32, tag=f"colv{d}")
                nc.vector.tensor_reduce(col_vec[:], cdist[:], axis=X, op=OP.add)
                # cross-partition sum via a DRAM bounce + free-axis reduce
                # (engines cannot mix start partitions; DMA relayouts freely)
                both = accp.tile([128, 2], fp32, tag=f"both{d}")
                nc.vector.tensor_copy(both[:, 0:1], geo_vec[:])
                nc.vector.tensor_copy(both[:, 1:2], col_vec[:])
                bothd = idxdp.tile([128, 2], fp32, tag=f"bothd{d}")
                nc.sync.dma_start(bothd[:], both[:])
                bothr = smallp.tile([1, 2, 128], fp32, tag=f"bothr{d}")
                nc.sync.dma_start(bothr[:], bothd[:].rearrange("p k -> k p").unsqueeze(0))
                nc.vector.tensor_reduce(
                    staging[0:1, 2 * d:2 * d + 2], bothr[:], axis=X, op=OP.add
                )

            # norm^2 max over full x
            nsq = featp.tile([128, C // 128], fp32, tag="nsq")
            nc.sync.dma_start(nsq[:], ins["xnormsq"][:])
            nmax = accp.tile([128, 1], fp32, tag="nmax")
            nc.vector.tensor_reduce(nmax[:], nsq[:], axis=X, op=OP.max)
            nmaxd = idxdp.tile([128, 1], fp32, tag="nmaxd")
            nc.sync.dma_start(nmaxd[:], nmax[:])
            nmaxr = smallp.tile([1, 1, 128], fp32, tag="nmaxr")
            nc.sync.dma_start(nmaxr[:], nmaxd[:].rearrange("p k -> k p").unsqueeze(0))
            nc.vector.tensor_reduce(staging[0:1, 4:5], nmaxr[:], axis=X, op=OP.max)

            nc.sync.dma_start(partials[:], staging[:])

    nc.compile()
    return nc


def make_in_maps(x, y, R=2048, C=8192):
    """Host-side sharding: per-core input dict."""
    in_maps = []
    shards = N_CORES // B
    for c in range(N_CORES):
        b, s = divmod(c, shards)
        sl = slice(s * R, (s + 1) * R)
        xs, ys = x[b][sl], y[b][sl]
        xnormsq = (x[b][:, :3].astype(np.float32) ** 2).sum(1, dtype=np.float32)
        m = {
            "stat0": _stat_feats(xs), "mov0": _mov_feats(y[b][:C]),
            "rows0": _rows_t(xs), "gsrc0": np.ascontiguousarray(y[b][:C]).astype(np.float32),
            "stat1": _stat_feats(ys), "mov1": _mov_feats(x[b][:C]),
            "rows1": _rows_t(ys), "gsrc1": np.ascontiguousarray(x[b][:C]).astype(np.float32),
            "xnormsq": np.ascontiguousarray(xnormsq[:C].reshape(128, C // 128)),
            "iotaf": np.broadcast_to(
                np.arange(C // 64, dtype=np.float32)[None, :], (128, C // 64)
            ).copy(),
        }
        in_maps.append(m)
    return in_maps


def combine_partials(parts):
    """parts: list of 8 arrays [1,8] -> (total, geo_part, color_part)."""
    shards = N_CORES // B
    loss_x2y = 0.0
    loss_y2x = 0.0
    loss_color = 0.0
    for b in range(B):
        cores = [parts[b * shards + s][0] for s in range(shards)]
        norm = float(np.sqrt(max(p[4] for p in cores)))
        loss_x2y += sum(float(p[0]) for p in cores) / norm
        loss_y2x += sum(float(p[2]) for p in cores) / norm
        loss_color += sum(float(p[1]) + float(p[3]) for p in cores)
    loss_x2y /= B * N
    loss_y2x /= B * M
    loss_color = loss_color / (B * N)  # N == M; both direction means share it
    total = ALPHA * loss_x2y + ALPHA * loss_y2x + (1.0 - ALPHA) * loss_color
    geo_part = ALPHA * loss_x2y + ALPHA * loss_y2x
    color_part = (1.0 - ALPHA) * loss_color
    return (
        np.float32(total),
        np.float32(geo_part),
        np.float32(color_part),
    )


_PROGRAM_CACHE = {}


def kernel(x, y):
    from concourse.bass_utils import run_bass_kernel_spmd

    x = np.asarray(x, dtype=np.float32)
    y = np.asarray(y, dtype=np.float32)
    key = "full"
    if key not in _PROGRAM_CACHE:
        _PROGRAM_CACHE[key] = build_program()
    nc = _PROGRAM_CACHE[key]
    in_maps = make_in_maps(x, y)
    res = run_bass_kernel_spmd(nc, in_maps, core_ids=list(range(N_CORES)))
    parts = [res.results[c]["partials"] for c in range(N_CORES)]
    return combine_partials(parts)


if __name__ == "__main__":
    xs = np.load("/tmp/x.npy")
    ys = np.load("/tmp/y.npy")
    out = kernel(xs, ys)
    print("kernel:", [float(v) for v in out])
